# revision 26
# baseline (speedup 1.0000x reference)
"""Trainium2 Bass kernel for nn_Encoder_76768245448827 (sparse_attention).

v7: f16 residual stream; row-tiled K=32 score matmuls (4 concurrent PE
strips); single activation table (exp/square/ln only — mish via
x*(1 - 2/((1+e^x)^2 + 1)), no tanh/softplus tables); Pool-engine offload
(LN rsqrt chain, squares, mish multiply); batch-staggered emission;
softmax denominators via ones-column in V; per-column 1/denom broadcast
through a block-ones matmul.
"""
import math

import numpy as np

import concourse.bass as bass
import concourse.mybir as mybir
import concourse.tile as tile
from concourse import bacc
from concourse.bass_utils import run_bass_kernel_spmd
from concourse.masks import make_identity

F32 = mybir.dt.float32
F16 = mybir.dt.float16
U32 = mybir.dt.uint32
AF = mybir.ActivationFunctionType
ALU = mybir.AluOpType
AX = mybir.AxisListType

L, HEADS, TOPK, NFFN, H = 4, 8, 32, 2, 256
B, M, D = 16, 512, 32
NCORES = 8
BPC = B // NCORES
SCALE = 1.0 / math.sqrt(D)
G = H // 128   # feature groups (2)
MT = M // 128  # token tiles (4)
EW_EPS = 1e-5
RSQRT_MAGIC = 0x5F3759DF
RECIP_MAGIC = 0x7EF311C3


def build(trivial_affine=False):
    nc = bacc.Bacc(name="encoder76v3")

    node = nc.declare_dram_parameter("node", [BPC, M, H], F32, isOutput=False)
    edge = nc.declare_dram_parameter("edge", [BPC, M, M], F32, isOutput=False)
    wd, bd = {}, {}
    for i in range(L):
        for nm in ("q", "k", "v", "o", "1", "2"):
            wd[nm, i] = nc.declare_dram_parameter(f"w{nm}{i}", [H, H], F16,
                                                  isOutput=False)
        for nm in ("q", "k", "o", "1", "2"):
            bd[nm, i] = nc.declare_dram_parameter(f"b{nm}{i}", [H], F32,
                                                  isOutput=False)
        bd["v", i] = nc.declare_dram_parameter(f"bv{i}", [H], F16, isOutput=False)
    lna_d = nc.declare_dram_parameter("lna", [H], F32, isOutput=False)
    lnb_d = nc.declare_dram_parameter("lnb", [H], F32, isOutput=False)
    blk_d = nc.declare_dram_parameter("blk8", [8, H], F16, isOutput=False)
    out = nc.declare_dram_parameter("out", [BPC, M, H], F32, isOutput=True)

    from contextlib import ExitStack
    with tile.TileContext(nc) as tc, ExitStack() as ctx:
        wpool = ctx.enter_context(tc.tile_pool(name="wpool", bufs=1))
        lwpool = ctx.enter_context(tc.tile_pool(name="lwpool", bufs=2))
        xpool = ctx.enter_context(tc.tile_pool(name="xpool", bufs=2))
        ewpool = ctx.enter_context(tc.tile_pool(name="ewpool", bufs=1))
        work = ctx.enter_context(tc.tile_pool(name="work", bufs=2))
        qkpool = ctx.enter_context(tc.tile_pool(name="qkpool", bufs=1))
        vtpool = ctx.enter_context(tc.tile_pool(name="vtpool", bufs=1))
        tbpool = ctx.enter_context(tc.tile_pool(name="tbpool", bufs=2))
        ebpool = ctx.enter_context(tc.tile_pool(name="ebpool", bufs=1))
        mpool = ctx.enter_context(tc.tile_pool(name="mpool", bufs=1))
        stat_pool = ctx.enter_context(tc.tile_pool(name="stat", bufs=2))
        dram = ctx.enter_context(tc.tile_pool(name="dram", bufs=2, space="DRAM"))
        ps_sc = ctx.enter_context(tc.tile_pool(name="ps_sc", bufs=2, space="PSUM"))
        ps_attn = ctx.enter_context(tc.tile_pool(name="ps_attn", bufs=2, space="PSUM"))
        ps_proj = ctx.enter_context(tc.tile_pool(name="ps_proj", bufs=2, space="PSUM"))

        # ---- constants ----
        ident = wpool.tile([128, 128], F32, tag="ident")
        make_identity(nc, ident)
        ident16 = wpool.tile([128, 128], F16, tag="ident16")
        nc.vector.tensor_copy(ident16, ident)
        ones_col16 = wpool.tile([128, 1], F16, tag="ones_col16")
        nc.vector.memset(ones_col16, 1.0)
        ones1 = wpool.tile([1, 128], F16, tag="ones1")
        nc.vector.memset(ones1, 1.0)
        magic_t = wpool.tile([128, MT], U32, tag="magic")
        nc.vector.memset(magic_t, RSQRT_MAGIC)
        lnA = wpool.tile([128, G], F32, tag="lnA")
        nc.sync.dma_start(out=lnA, in_=bass.AP(tensor=lna_d, offset=0,
                                               ap=[[1, 128], [128, G]]))
        lnB = wpool.tile([128, G], F32, tag="lnB")
        nc.sync.dma_start(out=lnB, in_=bass.AP(tensor=lnb_d, offset=0,
                                               ap=[[1, 128], [128, G]]))
        blk8 = wpool.tile([8, H], F16, tag="blk8")
        nc.sync.dma_start(out=blk8, in_=blk_d[:, :])
        one1x1 = wpool.tile([1, 1], F32, tag="one1x1")
        nc.vector.memset(one1x1, 1.0)
        ones_col32 = wpool.tile([128, 1], F32, tag="ones_col32")
        nc.vector.memset(ones_col32, 1.0)

        def warm(dep_ap, f32=False):
            """Tiny matmul reading a chain intermediate: keeps PE_HAM at 8/8
            through otherwise PE-idle serial chains."""
            wp = ps_proj.tile([1, M], F32, tag="proj", name="warm")
            lhs = ones_col32 if f32 else ones_col16
            nc.tensor.matmul(wp, lhs, dep_ap, start=True, stop=True)
        rmagic8 = wpool.tile([8, M], U32, tag="rmagic8")
        nc.vector.memset(rmagic8, RECIP_MAGIC)
        rmagic1 = wpool.tile([128, 1], U32, tag="rmagic1")
        nc.vector.memset(rmagic1, RECIP_MAGIC)

        def magic_recip(pool, dn, shape, tag, magic, iters=2):
            """1/dn via bit-trick + Newton; returns f32 tile."""
            r = pool.tile(shape, U32, tag=f"{tag}_r0", bufs=1)
            nc.vector.tensor_sub(r, magic[0:shape[0], 0:shape[1]],
                                 dn.bitcast(U32))
            r = r.bitcast(F32)
            for j in range(iters):
                a = pool.tile(shape, F32, tag=f"{tag}_a{j}", bufs=1)
                nc.vector.tensor_mul(a, dn, r)
                bns = pool.tile(shape, F32, tag=f"{tag}_b{j}", bufs=1)
                nc.vector.tensor_scalar(bns, a, -1.0, 2.0, op0=ALU.mult,
                                        op1=ALU.add)
                rn = pool.tile(shape, F32, tag=f"{tag}_rn{j}", bufs=1)
                nc.vector.tensor_mul(rn, r, bns)
                r = rn
            return r

        def load_layer_weights(i):
            Wl, Bl = {}, {}
            for nm in ("q", "k", "v", "o", "1", "2"):
                t0 = lwpool.tile([128, H], F16, tag=f"w{nm}_0", name=f"w{nm}_0")
                t1 = lwpool.tile([128, H], F16, tag=f"w{nm}_1", name=f"w{nm}_1")
                nc.sync.dma_start(out=t0, in_=wd[nm, i][0:128, :])
                nc.sync.dma_start(out=t1, in_=wd[nm, i][128:256, :])
                Wl[nm] = (t0, t1)
            for nm in ("q", "k", "o", "1", "2"):
                t = lwpool.tile([128, G], F32, tag=f"b{nm}", name=f"b{nm}")
                nc.sync.dma_start(out=t, in_=bass.AP(tensor=bd[nm, i], offset=0,
                                                     ap=[[1, 128], [128, G]]))
                Bl[nm] = t
            bvr = lwpool.tile([1, H], F16, tag="bv_row", name="bv_row")
            nc.sync.dma_start(out=bvr, in_=bd["v", i][:].rearrange("(o h) -> o h", o=1))
            Bl["v"] = bvr
            return Wl, Bl

        # ---- inputs -> feature-major f16 x[b][g] [128, 512] ----
        xT = {}
        for b in range(BPC):
            for g in range(G):
                xT[b, g] = xpool.tile([128, M], F16, tag=f"x_{b}_{g}", name="x0")
            for mt in range(MT):
                t = work.tile([128, H], F32, tag="xin", name="xin")
                nc.sync.dma_start(out=t, in_=node[b, 128 * mt:128 * (mt + 1), :])
                for g in range(G):
                    tp = ps_proj.tile([128, 128], F32, tag="proj", name="tps")
                    nc.tensor.transpose(tp, t[:, 128 * g:128 * (g + 1)], ident)
                    nc.vector.tensor_copy(xT[b, g][:, 128 * mt:128 * (mt + 1)], tp)

        # ---- edges: exact top-32 -> normalize -> transpose -> f16 ----
        ewnT = {}

        def emit_edges(b):
            for nt in range(MT):
                ewnT[b, nt] = ewpool.tile([128, M], F16, tag=f"ewnT_{b}_{nt}",
                                          name="ewnT")
            for mt in range(MT):
                e = work.tile([128, M], F32, tag="edge_in")
                nc.sync.dma_start(out=e, in_=edge[b, 128 * mt:128 * (mt + 1), :])
                scratch = work.tile([128, M], F32, tag="topk_scratch")
                maxes = work.tile([128, 8], F32, tag="topk_max")
                cur = e
                for it in range(TOPK // 8):
                    nc.vector.max(out=maxes, in_=cur)
                    nc.vector.match_replace(out=scratch, in_to_replace=maxes,
                                            in_values=cur, imm_value=0.0)
                    cur = scratch
                    if it == 1:
                        warm(scratch, f32=True)
                ew = work.tile([128, M], F32, tag="ew")
                nc.gpsimd.tensor_sub(ew, e, scratch)
                rs = work.tile([128, 1], F32, tag="ew_rs")
                nc.vector.reduce_sum(rs, ew, axis=AX.X)
                rse = work.tile([128, 1], F32, tag="ew_rse")
                nc.vector.tensor_scalar(rse, rs, EW_EPS, None, op0=ALU.add)
                rec = magic_recip(work, rse, [128, 1], "ewrec", rmagic1,
                                  iters=3)
                ewn = work.tile([128, M], F32, tag="ewn", name="ewn")
                nc.vector.tensor_scalar(ewn, ew, rec, SCALE, op0=ALU.mult, op1=ALU.mult)
                for nt in range(MT):
                    tp = ps_proj.tile([128, 128], F32, tag="proj", name="tps")
                    nc.tensor.transpose(tp, ewn[:, 128 * nt:128 * (nt + 1)], ident)
                    nc.vector.tensor_copy(
                        ewnT[b, nt][:, 128 * mt:128 * (mt + 1)], tp)

        # ---- layernorm (per batch): replaces the residual stream ----
        def layernorm(b):
            # squares on Pool engine
            x2 = []
            for g in range(G):
                s = stat_pool.tile([128, M], F16, tag="x2")
                nc.gpsimd.tensor_mul(s, xT[b, g], xT[b, g])
                x2.append(s)
            sum_ps = ps_proj.tile([1, M], F32, tag="proj", name="stats")
            nc.tensor.matmul(sum_ps, ones_col16, xT[b, 0], start=True, stop=False)
            nc.tensor.matmul(sum_ps, ones_col16, xT[b, 1], start=False, stop=True)
            sq_ps = ps_proj.tile([1, M], F32, tag="proj", name="stats")
            nc.tensor.matmul(sq_ps, ones_col16, x2[0], start=True, stop=False)
            nc.tensor.matmul(sq_ps, ones_col16, x2[1], start=False, stop=True)
            srow01 = stat_pool.tile([1, 2 * M], F32, tag="srow01", bufs=1)
            nc.scalar.copy(srow01[:, 0:M], sum_ps)
            nc.scalar.copy(srow01[:, M:2 * M], sq_ps)
            # rows -> partition-major [128, 8] via K=1 matmuls
            comb_ps = ps_proj.tile([128, 2 * MT], F32, tag="proj", name="compT")
            for c in range(2 * MT):
                nc.tensor.transpose(comb_ps[:, c:c + 1],
                                    srow01[:, 128 * c:128 * (c + 1)],
                                    ident[0:1, 0:1])
            comb = stat_pool.tile([128, 2 * MT], F32, tag="comb")
            nc.vector.tensor_copy(comb, comb_ps)
            sum4 = comb[:, 0:MT]
            sq4 = comb[:, MT:2 * MT]
            t1 = stat_pool.tile([128, MT], F32, tag="ln_t1")
            nc.vector.tensor_mul(t1, sum4, sum4)
            sq255 = stat_pool.tile([128, MT], F32, tag="ln_sq255")
            nc.vector.tensor_scalar(sq255, sq4, 1.0 / (H - 1), None, op0=ALU.mult)
            var = stat_pool.tile([128, MT], F32, tag="ln_var")
            nc.vector.scalar_tensor_tensor(var, t1, -1.0 / (H * (H - 1)), sq255,
                                           op0=ALU.mult, op1=ALU.add)
            sh = stat_pool.tile([128, MT], U32, tag="ln_sh")
            nc.vector.tensor_scalar(sh, var.bitcast(U32), 1, None,
                                    op0=ALU.logical_shift_right)
            r_u = stat_pool.tile([128, MT], U32, tag="ln_ru")
            nc.vector.tensor_sub(r_u, magic_t, sh)
            r = r_u.bitcast(F32)
            for it in range(2):
                rr = stat_pool.tile([128, MT], F32, tag="ln_rr")
                nc.vector.tensor_mul(rr, r, r)
                rrv = stat_pool.tile([128, MT], F32, tag="ln_rrv")
                nc.vector.tensor_mul(rrv, rr, var)
                f = stat_pool.tile([128, MT], F32, tag="ln_f")
                nc.vector.tensor_scalar(f, rrv, -0.5, 1.5, op0=ALU.mult, op1=ALU.add)
                rn = stat_pool.tile([128, MT], F32, tag="ln_rn")
                nc.vector.tensor_mul(rn, r, f)
                r = rn
            comb16 = stat_pool.tile([128, 2 * MT], F16, tag="comb16")
            nc.vector.tensor_copy(comb16[:, 0:MT], r)
            nc.vector.scalar_tensor_tensor(comb16[:, MT:2 * MT], sum4, -1.0 / H,
                                           r, op0=ALU.mult, op1=ALU.mult)
            # comb16 [128,8] -> rows [8,128] via PE transpose, then
            # broadcast rows across partitions via K=1 ones matmuls
            rows_ps = ps_proj.tile([8, 128], F16, tag="proj", name="rowsT")
            nc.tensor.transpose(rows_ps, comb16, ident16)
            rows16 = stat_pool.tile([8, 128], F16, tag="rows16", bufs=1)
            nc.vector.tensor_copy(rows16, rows_ps)
            rowflat = stat_pool.tile([1, 8 * 128], F16, tag="rowflat", bufs=1)
            nc.sync.dma_start(out=rowflat, in_=rows16)
            rbnb = stat_pool.tile([128, 2, M], F16, tag="ln_rbnb", bufs=1)
            for w in range(2):
                rb_ps = ps_attn.tile([128, M], F32, tag="attn", name="ln_rb")
                nc.tensor.matmul(rb_ps, ones1, rowflat[:, M * w:M * (w + 1)],
                                 start=True, stop=True)
                nc.scalar.copy(rbnb[:, w, :], rb_ps)
            warm(rbnb[:, 0, :])
            for g in range(G):
                t_ = stat_pool.tile([128, M], F16, tag="ln_t", bufs=1)
                nc.vector.tensor_mul(t_, xT[b, g], rbnb[:, 0, :])
                xnew = xpool.tile([128, M], F16, tag=f"x_{b}_{g}", name="xln")
                if trivial_affine:
                    nc.vector.tensor_add(xnew, t_, rbnb[:, 1, :])
                else:
                    t2 = stat_pool.tile([128, M], F16, tag="ln_t2", bufs=1)
                    nc.vector.tensor_add(t2, t_, rbnb[:, 1, :])
                    nc.vector.tensor_scalar(xnew, t2, lnA[:, g:g + 1],
                                            lnB[:, g:g + 1],
                                            op0=ALU.mult, op1=ALU.add)
                xT[b, g] = xnew

        # ---- mish helpers (exp/square/ln only; th = 1 - 2/((1+u)^2+1)) ----
        # quarters: list of (psum_ap, bias_ap); writes am tile [128, 2048] f16
        def mish_block(tag, quarters):
            u = mpool.tile([128, 4 * M], F16, tag="mish_u", name="mish_u")
            z = mpool.tile([128, 4 * M], F16, tag="mish_z", name="mish_z")
            for qi, (ps, bias) in enumerate(quarters):
                nc.scalar.activation(u[:, M * qi:M * (qi + 1)], ps, AF.Exp,
                                     bias=bias)
                nc.vector.tensor_scalar(z[:, M * qi:M * (qi + 1)], ps, bias,
                                        None, op0=ALU.add)
            sp = mpool.tile([128, 4 * M], F16, tag="mish_sp", name="mish_sp")
            nc.scalar.activation(sp, u, AF.Ln, bias=1.0)
            th = mpool.tile([128, 4 * M], F16, tag="mish_th", name="mish_th")
            nc.scalar.activation(th, sp, AF.Tanh)
            ams = []
            for half in range(2):
                amh = mpool.tile([128, 2 * M], F16, tag=f"{tag}{half}",
                                 name=tag)
                nc.vector.tensor_mul(amh, th[:, 2 * M * half:2 * M * (half + 1)],
                                     z[:, 2 * M * half:2 * M * (half + 1)])
                ams.append(amh)
            return ams

        # ---- layers (software-pipelined: b1 trails b0 by one phase) ----
        def mish_block(tag, b, quarters):
            u = mpool.tile([128, 2 * M], F16, tag=f"mish_u{b}", name="mish_u")
            z = mpool.tile([128, 2 * M], F16, tag=f"mish_z{b}", name="mish_z")
            for qi, (ps, bias) in enumerate(quarters):
                nc.scalar.activation(u[:, M * qi:M * (qi + 1)], ps, AF.Exp,
                                     bias=bias)
                nc.vector.tensor_scalar(z[:, M * qi:M * (qi + 1)], ps, bias,
                                        None, op0=ALU.add)
            sp = mpool.tile([128, 2 * M], F16, tag=f"mish_sp{b}", name="mish_sp")
            nc.scalar.activation(sp, u, AF.Ln, bias=1.0)
            warm(sp[:, 0:M])
            th = mpool.tile([128, 2 * M], F16, tag=f"mish_th{b}", name="mish_th")
            nc.scalar.activation(th, sp, AF.Tanh)
            warm(th[:, 0:M])
            am = mpool.tile([128, 2 * M], F16, tag=f"{tag}_{b}", name=tag)
            nc.vector.tensor_mul(am, th, z)
            return am

        carry = None
        for i in range(NL):
            W, BIAS = load_layer_weights(i)
            QQ, KK, VT, EB, CAT, DEN, CT, AMF1 = {}, {}, {}, {}, {}, {}, {}, {}

            def p0(b, W=W, BIAS=BIAS):
                layernorm(b)

            def p1(b, W=W, BIAS=BIAS, QQ=QQ, KK=KK, VT=VT):
                for nm, store in (("q", QQ), ("k", KK)):
                    for Q in range(2):
                        qsl = bass.ts(Q, 128)
                        ps = ps_proj.tile([128, M], F32, tag="proj", name="qk_ps")
                        nc.tensor.matmul(ps, W[nm][0][:, qsl], xT[b, 0],
                                         start=True, stop=False)
                        nc.tensor.matmul(ps, W[nm][1][:, qsl], xT[b, 1],
                                         start=False, stop=True)
                        qt = qkpool.tile([128, M], F16, tag=f"{nm}{Q}_{b}",
                                         name=f"{nm}{Q}")
                        nc.scalar.activation(qt, ps, AF.Identity,
                                             bias=BIAS[nm][:, Q:Q + 1])
                        store[b, Q] = qt
                for nt in range(MT):
                    ntsl = bass.ts(nt, 128)
                    ps = ps_proj.tile([128, H], F32, tag="proj", name="v_ps")
                    nc.tensor.matmul(ps, xT[b, 0][:, ntsl], W["v"][0],
                                     start=True, stop=False)
                    nc.tensor.matmul(ps, xT[b, 1][:, ntsl], W["v"][1],
                                     start=False, stop=trivial_affine)
                    if not trivial_affine:
                        nc.tensor.matmul(ps, ones1, BIAS["v"],
                                         start=False, stop=True)
                    vt = vtpool.tile([128, HEADS, D + 1], F16, tag=f"V{nt}_{b}",
                                     name="vt")
                    nc.scalar.copy(vt[:, :, 0:D],
                                   ps.rearrange("p (h d) -> p h d", h=HEADS))
                    nc.vector.memset(vt[:, :, D:D + 1], 1.0)
                    VT[b, nt] = vt

            def p2(b, i=i, QQ=QQ, KK=KK, EB=EB):
                if i == 0:
                    emit_edges(b)
                for nt in range(MT):
                    for Q in range(2):
                        tbQ = tbpool.tile([128, 4 * M], F16, tag="tb", name="tb")
                        for j in range(2):
                            sps = ps_sc.tile([128, 2 * M], F32, tag="sc", name="sc")
                            for hh in range(2):
                                s = 2 * j + hh
                                nc.tensor.matmul(
                                    sps[:, bass.ts(hh, M)],
                                    KK[b, Q][32 * s:32 * (s + 1), bass.ts(nt, 128)],
                                    QQ[b, Q][32 * s:32 * (s + 1), :],
                                    start=True, stop=True,
                                    tile_position=(32 * s, 0))
                            nc.vector.tensor_tensor(
                                tbQ[:, 2 * M * j:2 * M * (j + 1)]
                                .rearrange("p (r m) -> p r m", r=2),
                                sps.rearrange("p (r m) -> p r m", r=2),
                                ewnT[b, nt].rearrange("p (o m) -> p o m", o=1)
                                .broadcast_to([128, 2, M]),
                                op=ALU.mult)
                        ebt = ebpool.tile([128, 4 * M], F16, tag=f"eb_{nt}_{Q}",
                                          name="eb")
                        nc.scalar.activation(ebt, tbQ, AF.Exp)
                        EB[b, nt, Q] = ebt

            def p3(b, VT=VT, EB=EB, CAT=CAT, DEN=DEN):
                cat = [work.tile([128, M], F32, tag=f"cat{g}_{b}", name="cat")
                       for g in range(G)]
                den8 = work.tile([8, M], F32, tag=f"den8_{b}", name="den8")
                for h in range(HEADS):
                    Q, hq = h // 4, h % 4
                    aps = ps_attn.tile([D + 1, M], F32, tag="attn", name="attn")
                    for nt in range(MT):
                        nc.tensor.matmul(aps, VT[b, nt][:, h, :],
                                         EB[b, nt, Q][:, bass.ts(hq, M)],
                                         start=(nt == 0), stop=(nt == MT - 1))
                    avsb = work.tile([D + 1, M], F32, tag="avsb", name="avsb")
                    nc.scalar.copy(avsb, aps)
                    nc.sync.dma_start(out=cat[Q][D * hq:D * (hq + 1), :],
                                      in_=avsb[0:D, :])
                    nc.sync.dma_start(out=den8[h:h + 1, :], in_=avsb[D:D + 1, :])
                CAT[b] = cat
                DEN[b] = den8

            def p4(b, CAT=CAT, DEN=DEN, CT=CT):
                rec = magic_recip(work, DEN[b], [8, M], "denrec", rmagic8,
                                  iters=1)
                r16 = work.tile([8, M], F16, tag=f"r16_{b}", name="r16")
                nc.vector.tensor_copy(r16, rec)
                ct = []
                for g in range(G):
                    rb_ps = ps_proj.tile([128, M], F32, tag="proj", name="rb_ps")
                    nc.tensor.matmul(rb_ps, blk8[:, bass.ts(g, 128)], r16,
                                     start=True, stop=True)
                    c = work.tile([128, M], F16, tag=f"ct{g}_{b}", name="ct")
                    nc.vector.tensor_mul(c, CAT[b][g], rb_ps)
                    ct.append(c)
                CT[b] = ct

            def p5(b, W=W, BIAS=BIAS, CT=CT):
                oq = []
                for gout in range(G):
                    ps = ps_proj.tile([128, M], F32, tag="proj", name="o_ps")
                    osl = bass.ts(gout, 128)
                    nc.tensor.matmul(ps, W["o"][0][:, osl], CT[b][0],
                                     start=True, stop=False)
                    nc.tensor.matmul(ps, W["o"][1][:, osl], CT[b][1],
                                     start=False, stop=True)
                    oq.append((ps, BIAS["o"][:, gout:gout + 1]))
                am = mish_block("am_o", b, oq)
                for g in range(G):
                    xnew = xpool.tile([128, M], F16, tag=f"x_{b}_{g}", name="xres")
                    nc.vector.tensor_add(xnew, xT[b, g],
                                         am[:, M * g:M * (g + 1)])
                    xT[b, g] = xnew

            def p6(b):
                layernorm(b)

            def p7(b, W=W, BIAS=BIAS, AMF1=AMF1):
                f1q = []
                for gout in range(G):
                    ps = ps_proj.tile([128, M], F32, tag="proj", name="f1_ps")
                    osl = bass.ts(gout, 128)
                    nc.tensor.matmul(ps, W["1"][0][:, osl], xT[b, 0],
                                     start=True, stop=False)
                    nc.tensor.matmul(ps, W["1"][1][:, osl], xT[b, 1],
                                     start=False, stop=True)
                    f1q.append((ps, BIAS["1"][:, gout:gout + 1]))
                AMF1[b] = mish_block("am_f1", b, f1q)

            def p8(b, W=W, BIAS=BIAS, AMF1=AMF1):
                f2q = []
                for gout in range(G):
                    ps = ps_proj.tile([128, M], F32, tag="proj", name="f2_ps")
                    osl = bass.ts(gout, 128)
                    nc.tensor.matmul(ps, W["2"][0][:, osl], AMF1[b][:, 0:M],
                                     start=True, stop=False)
                    nc.tensor.matmul(ps, W["2"][1][:, osl], AMF1[b][:, M:2 * M],
                                     start=False, stop=True)
                    f2q.append((ps, BIAS["2"][:, gout:gout + 1]))
                am = mish_block("am_f2", b, f2q)
                for g in range(G):
                    xnew = xpool.tile([128, M], F16, tag=f"x_{b}_{g}", name="xres2")
                    nc.vector.tensor_add(xnew, xT[b, g],
                                         am[:, M * g:M * (g + 1)])
                    xT[b, g] = xnew

            phases = [p0, p1, p2, p3, p4, p5, p6, p7, p8]
            for k in range(9):
                phases[k](0)
                if k >= 2:
                    phases[k - 2](1)
                elif carry is not None:
                    carry[k](1)
            carry = (phases[7], phases[8])
        # ---- output (b0 emitted before b1's final carried phase) ----
        def emit_out(b):
            for mt in range(MT):
                ot_sb = work.tile([128, H], F32, tag="out_sb")
                for g in range(G):
                    tp = ps_sc.tile([128, 128], F16, tag="sc", name="tps16")
                    nc.tensor.transpose(tp, xT[b, g][:, bass.ts(mt, 128)],
                                        ident16)
                    nc.scalar.copy(ot_sb[:, bass.ts(g, 128)], tp)
                nc.sync.dma_start(out=out[b, 128 * mt:128 * (mt + 1), :], in_=ot_sb)

        emit_out(0)
        carry[0](1)
        carry[1](1)
        emit_out(1)

    nc.finalize()
    return nc


_NC_CACHE = {}
NL = L
TRACE = False
LAST_EXEC_NS = None
LAST_RESULTS = None


def _get_nc(trivial_affine):
    key = ("nc", trivial_affine)
    if key not in _NC_CACHE:
        _NC_CACHE[key] = build(trivial_affine)
    return _NC_CACHE[key]


def _prep_weights(attn_W, attn_b, ffn_W, ffn_b, ln_a, ln_b):
    ws = {}
    for i in range(L):
        ws[f"wq{i}"] = attn_W[i, 0].T.astype(np.float16)
        ws[f"wk{i}"] = attn_W[i, 1].T.astype(np.float16)
        ws[f"wv{i}"] = attn_W[i, 2].T.astype(np.float16)
        ws[f"wo{i}"] = attn_W[i, 3].T.astype(np.float16)
        ws[f"w1{i}"] = ffn_W[i, 0].T.astype(np.float16)
        ws[f"w2{i}"] = ffn_W[i, 1].T.astype(np.float16)
        ws[f"bq{i}"] = attn_b[i, 0].astype(np.float32)
        ws[f"bk{i}"] = attn_b[i, 1].astype(np.float32)
        ws[f"bv{i}"] = attn_b[i, 2].astype(np.float16)
        ws[f"bo{i}"] = attn_b[i, 3].astype(np.float32)
        ws[f"b1{i}"] = ffn_b[i, 0].astype(np.float32)
        ws[f"b2{i}"] = ffn_b[i, 1].astype(np.float32)
    ws["lna"] = ln_a.astype(np.float32)
    ws["lnb"] = ln_b.astype(np.float32)
    # blk8[h, 128g + p] = 1 iff h == 4g + p//32  (per-column head recip bcast)
    blk = np.zeros((8, H), np.float16)
    for h in range(8):
        g, hq = h // 4, h % 4
        blk[h, 128 * g + 32 * hq:128 * g + 32 * (hq + 1)] = 1.0
    ws["blk8"] = blk
    return ws


def kernel(node_features, edge_features, masks, attn_W, attn_b, ffn_W, ffn_b,
           ln_a, ln_b):
    node_features = np.asarray(node_features, dtype=np.float32)
    edge_features = np.asarray(edge_features, dtype=np.float32)
    ws = _prep_weights(np.asarray(attn_W), np.asarray(attn_b),
                       np.asarray(ffn_W), np.asarray(ffn_b),
                       np.asarray(ln_a), np.asarray(ln_b))
    trivial = (np.all(np.asarray(ln_a) == 1.0) and np.all(np.asarray(ln_b) == 0.0)
               and np.all(np.asarray(attn_b)[:, 2] == 0.0))
    nc = _get_nc(bool(trivial))
    in_maps = []
    for c in range(NCORES):
        m = {"node": node_features[BPC * c:BPC * (c + 1)],
             "edge": edge_features[BPC * c:BPC * (c + 1)]}
        m.update(ws)
        in_maps.append(m)
    res = run_bass_kernel_spmd(nc, in_maps, list(range(NCORES)), trace=TRACE)
    global LAST_EXEC_NS, LAST_RESULTS
    LAST_EXEC_NS = res.exec_time_ns
    LAST_RESULTS = res
    return np.concatenate([res.results[c]["out"] for c in range(NCORES)], axis=0)


if __name__ == "__main__":
    build()
    print("build OK")
